# revision 22
# baseline (speedup 1.0000x reference)
"""Trainium2 Bass kernel for nn_CrossAttention_15006615733765 (raw Bass, no Tile).

Mathematical structure: the reference broadcasts a per-batch context vector
(B, CTX_DIM) to every spatial position before projecting to K/V.  All keys
within a batch are therefore identical, softmax over the key axis is exactly
uniform, and the attention output equals V itself.  The module collapses to

    out[b, c, h, w] = ((context[b] @ Wv) @ Wo + bo)[c]

independent of x, Wq and Wk (exact in infinite precision).  The kernel
computes the two small matmuls on the tensor engine and materializes the
broadcast output shard per core, sharding the 512 output channels across the
8 cores (64 each).

All on-device data is fp16: the 2e-2 rel-err budget dwarfs fp16 rounding
(~5e-4 measured), the Wv stream halves to 768 KB, matmuls avoid the 4x fp32
LOW_HIGH penalty, and the output store halves to 1.18 MB (the host unshard
upcasts to fp32).

Correctness notes learned on HW:
  * start=True (first_mm) clears the whole 2 KiB PSUM bank, racing any
    sibling accumulation group in the same bank.  All matmuls here use
    start=False; the DVE zeroes the result banks at body start instead
    (overwrite-on-clear / accumulate-on-set is then correct regardless of
    stale has_written state).
  * A dma_start always fires 16 semaphore increments, one per SDMA engine;
    for transfers with <16 descriptors the excess increments are padding
    that can land BEFORE the data descriptors on other engines.  Every
    gated load here therefore spans all 128 partitions (the small consts
    ride inside the one cw tensor; bias/ones/selector rows are embedded in
    its partition-0..3 columns).

Engine plan (raw Bass, hand-placed semaphores):
  Sync   : Wv chunks 0/2/4 (HWDGE), final broadcast store
  Scalar : Wv chunks 1/3/5 (HWDGE)
  GpSimd : single cw consts load (SWDGE, off the Wv queues)
  Tensor : HAM warmup -> stage1 (y1T direct: Wv chunks stationary, ctx
           streaming, so no PE transposes are needed) -> stage2 (+bias via
           a K=1 ones-row matmul) -> selector broadcast MMs
  Vector : PSUM bank zeroing, PSUM -> SBUF fp16 cast copies between PE
           stages, output row replication for 3 KiB DMA descriptors
The store has no explicit completion wait: the block-exit DRAIN on the sync
engine waits for the HWDGE queue, so the semaphore-reset epilogue overlaps
the output transfer instead of following it.
"""

import numpy as np

import concourse.bacc as bacc
import concourse.mybir as mybir
from concourse.bass_utils import run_bass_kernel_spmd

B, DIM, CTX_DIM = 4, 512, 768
H = W = 48
NPOS = H * W
NCORES = 8
CPC = DIM // NCORES  # 64 output channels per core
P = 128
KC = CTX_DIM // P  # 6 contraction chunks for stage 1
KD = DIM // P      # 4 contraction chunks for stage 2
ROW = B * CPC      # 256: one output row (all batches) per position
NDUP = 6           # replicated rows per partition -> 3 KiB descriptors
NREP = NPOS // (NDUP * P)  # 3 descriptor groups
F32 = mybir.dt.float32
F16 = mybir.dt.float16

# column offsets inside the packed consts tensor cw [P, CWN]
CTX0 = 0                  # ctx chunks: cw[p, CTX0 + k*B + b]
WO0 = CTX0 + KC * B       # Wo slice:   cw[p, WO0 + m*CPC + c]
SEL0 = WO0 + KD * CPC     # selector:   cw[r, SEL0 + b*P + q] = (r == b)
BO0 = SEL0 + B * P        # bias row:   cw[0, BO0 + c] = bo[c]
ONE0 = BO0 + CPC          # ones row:   cw[0, ONE0 + j] = 1.0
CWN = ONE0 + B

_CACHE: dict = {}


def _build_nc():
    nc = bacc.Bacc("TRN2", target_bir_lowering=False, debug=False, num_devices=NCORES)

    wvc = nc.dram_tensor("wvc", [P, KC * KD, P], F16, kind="ExternalInput")
    cwc = nc.dram_tensor("cwc", [P, CWN], F16, kind="ExternalInput")
    outd = nc.dram_tensor("outd", [NPOS, ROW], F16, kind="ExternalOutput")

    wv_sb = nc.alloc_sbuf_tensor("wv_sb", [P, KC * KD, P], F16).ap()
    cw_sb = nc.alloc_sbuf_tensor("cw_sb", [P, CWN], F16).ap()
    y1T_sb = nc.alloc_sbuf_tensor("y1T_sb", [P, KD, B], F16).ap()
    o4_sb = nc.alloc_sbuf_tensor("o4_sb", [B, CPC], F16).ap()
    rep_sb = nc.alloc_sbuf_tensor("rep_sb", [P, NDUP, ROW], F16).ap()
    warm_sb = nc.alloc_sbuf_tensor("warm_sb", [P, 512], F16).ap()

    py1T = nc.alloc_psum_tensor("py1T", [P, KD, B], F32).ap()
    po = nc.alloc_psum_tensor("po", [B, CPC], F32).ap()
    prep = nc.alloc_psum_tensor("prep", [P, B, CPC], F32).ap()
    pwarm = nc.alloc_psum_tensor("pwarm", [P, 512], F32).ap()

    from contextlib import ExitStack

    with ExitStack() as stack:
        s_wvs = stack.enter_context(nc.semaphore("s_wvs"))
        s_wvc = stack.enter_context(nc.semaphore("s_wvc"))
        s_cst = stack.enter_context(nc.semaphore("s_cst"))
        s_pz = stack.enter_context(nc.semaphore("s_pz"))
        s_mm = stack.enter_context(nc.semaphore("s_mm"))
        s_y1 = stack.enter_context(nc.semaphore("s_y1"))
        s_o4 = stack.enter_context(nc.semaphore("s_o4"))
        s_rep = stack.enter_context(nc.semaphore("s_rep"))
        s_out = stack.enter_context(nc.semaphore("s_out"))

        with nc.Block() as block:

            @block.sync
            def _(sync):
                for k in (0, 2, 4):
                    sync.dma_start(
                        out=wv_sb[:, k * KD:(k + 1) * KD, :],
                        in_=wvc[:, k * KD:(k + 1) * KD, :],
                    ).then_inc(s_wvs, 16)
                sync.wait_ge(s_rep, 1)
                # pos = r*(P*NDUP) + p*NDUP + d: each partition writes NREP
                # contiguous 3 KiB chunks (NDUP consecutive 512 B rows).
                out_view = outd.rearrange("(r p d) n -> p r (d n)", p=P, d=NDUP)
                src_view = (
                    rep_sb.rearrange("p d n -> p (d n)")[:, None, :]
                    .broadcast_to((P, NREP, NDUP * ROW))
                )
                # No completion wait: the block-exit DRAIN on the sync engine
                # waits for the HWDGE queue, so the semaphore-reset epilogue
                # overlaps the transfer.
                sync.dma_start(out=out_view, in_=src_view).then_inc(s_out, 16)

            @block.scalar
            def _(scalar):
                for k in (1, 3, 5):
                    scalar.dma_start(
                        out=wv_sb[:, k * KD:(k + 1) * KD, :],
                        in_=wvc[:, k * KD:(k + 1) * KD, :],
                    ).then_inc(s_wvc, 16)

            @block.gpsimd
            def _(g):
                g.dma_start(out=cw_sb[:], in_=cwc[:]).then_inc(s_cst, 16)

            @block.tensor
            def _(tensor):
                # HAM warmup: ungated dummy matmuls on scratch SBUF keep the
                # PE busy from boot so the 1.2->2.4 GHz unthrottle fires
                # while stage 1 is still load-gated.
                # stage 1: y1T[m*128+p, b] += Wv[k-chunk, m-chunk].T @ ctx
                # chunk.  Produces y1 already transposed -> no PE transposes.
                tensor.wait_ge(s_pz, 1)
                # HAM warmup: dummy matmuls on scratch SBUF keep the PE busy
                # so the 1.2->2.4 GHz unthrottle fires while stage 1 is
                # still load-gated (gated on s_pz to avoid concurrent
                # PE/DVE PSUM writes, which are fatal on this part).
                NWARM = 3
                for _w in range(NWARM):
                    nc.tensor.matmul(
                        pwarm[:],
                        warm_sb[:, 0:128],
                        warm_sb[:],
                        start=(_w == 0),
                        stop=(_w == NWARM - 1),
                    )
                tensor.wait_ge(s_cst, 16)
                ins = None
                for k in range(KC):
                    if k % 2 == 0:
                        tensor.wait_ge(s_wvs, 16 * (k // 2 + 1))
                    else:
                        tensor.wait_ge(s_wvc, 16 * (k // 2 + 1))
                    for m in range(KD):
                        ins = nc.tensor.matmul(
                            py1T[:, m, :],
                            wv_sb[:, k * KD + m, :],
                            cw_sb[:, CTX0 + k * B:CTX0 + (k + 1) * B],
                            start=False,
                            stop=(k == KC - 1),
                            skip_group_check=True,
                        )
                ins.then_inc(s_mm, 1)

                # stage 2: y[b, c] += y1T[m-chunk].T @ Wo[m-chunk, cols],
                # plus the bias via a K=1 ones-row matmul.
                tensor.wait_ge(s_y1, 1)
                for m in range(KD):
                    nc.tensor.matmul(
                        po[:],
                        y1T_sb[:, m, :],
                        cw_sb[:, WO0 + m * CPC:WO0 + (m + 1) * CPC],
                        start=False,
                        stop=False,
                        skip_group_check=True,
                    )
                ins = nc.tensor.matmul(
                    po[:],
                    cw_sb[0:1, ONE0:ONE0 + B],
                    cw_sb[0:1, BO0:BO0 + CPC],
                    start=False,
                    stop=True,
                    skip_group_check=True,
                )
                ins.then_inc(s_mm, 1)

                # selector broadcast: prep[p, b, :] = (y[b, :] + bo) for all p
                tensor.wait_ge(s_o4, 1)
                for b in range(B):
                    ins = nc.tensor.matmul(
                        prep[:, b, :],
                        cw_sb[0:B, SEL0 + b * P:SEL0 + (b + 1) * P],
                        o4_sb[:, :],
                        start=False,
                        stop=True,
                        skip_group_check=True,
                    )
                ins.then_inc(s_mm, 1)

            @block.vector
            def _(vector):
                # Zero the PSUM result banks so the PE matmuls never need
                # start=True (whose whole-bank clear races sibling groups).
                nc.vector.memset(py1T[:], 0.0)
                nc.vector.memset(po[:], 0.0)
                nc.vector.memset(prep[:], 0.0).then_inc(s_pz, 1)
                vector.wait_ge(s_mm, 1)
                nc.vector.tensor_copy(y1T_sb[:], py1T[:]).then_inc(s_y1, 1)
                vector.wait_ge(s_mm, 2)
                nc.vector.tensor_copy(o4_sb[:], po[:]).then_inc(s_o4, 1)
                vector.wait_ge(s_mm, 3)
                flat = prep[:].rearrange("p b c -> p (b c)")
                nc.vector.tensor_copy(rep_sb[:, 0, :], flat)
                # replicas 1..NDUP-1 in one SBUF->SBUF copy (broadcast source)
                nc.vector.tensor_copy(
                    rep_sb[:, 1:, :],
                    rep_sb[:, 0:1, :].broadcast_to((P, NDUP - 1, ROW)),
                ).then_inc(s_rep, 1)

    nc.compile()
    return nc


def _get_nc():
    if "nc" not in _CACHE:
        _CACHE["nc"] = _build_nc()
    return _CACHE["nc"]


def _prepare_in_maps(context, Wv, Wo, bo):
    context = np.asarray(context, dtype=np.float32)
    Wv = np.asarray(Wv, dtype=np.float32)
    Wo = np.asarray(Wo, dtype=np.float32)
    bo = np.asarray(bo, dtype=np.float32)

    # wvc[p, k*KD+m, c] = Wv[k*128+p, m*128+c]  (stationary operands)
    wvc = np.ascontiguousarray(
        Wv.astype(np.float16).reshape(KC, P, KD, P).transpose(1, 0, 2, 3)
        .reshape(P, KC * KD, P)
    )
    # ctx chunks: cw[p, CTX0 + k*B + b] = context[b, k*128+p]
    ctxc = (
        context.astype(np.float16).T.reshape(KC, P, B).transpose(1, 0, 2)
        .reshape(P, KC * B)
    )
    wo16 = Wo.astype(np.float16)
    bo16 = bo.astype(np.float16)

    in_maps = []
    for i in range(NCORES):
        cw = np.zeros((P, CWN), dtype=np.float16)
        cw[:, CTX0:CTX0 + KC * B] = ctxc
        # Wo slice: cw[p, WO0 + m*CPC + c] = Wo[m*128+p, i*CPC+c]
        cw[:, WO0:WO0 + KD * CPC] = (
            wo16[:, i * CPC:(i + 1) * CPC].reshape(KD, P, CPC)
            .transpose(1, 0, 2).reshape(P, KD * CPC)
        )
        for b in range(B):
            cw[b, SEL0 + b * P:SEL0 + (b + 1) * P] = 1.0
        cw[0, BO0:BO0 + CPC] = bo16[i * CPC:(i + 1) * CPC]
        cw[0, ONE0:ONE0 + B] = 1.0
        in_maps.append({"wvc": wvc, "cwc": cw})
    return in_maps


def _unshard(results):
    shards = np.stack([r["outd"] for r in results], axis=0)
    shards = shards.reshape(NCORES, NPOS, B, CPC)
    out = shards.transpose(2, 0, 3, 1).reshape(B, DIM, H, W)
    return np.ascontiguousarray(out.astype(np.float32))


def kernel(x, context, Wq, Wk, Wv, Wo, bo):
    del x, Wq, Wk
    nc = _get_nc()
    in_maps = _prepare_in_maps(context, Wv, Wo, bo)
    results = run_bass_kernel_spmd(nc, in_maps, list(range(NCORES))).results
    return _unshard(results)


# revision 23
# speedup vs baseline: 1.0482x; 1.0482x over previous
"""Trainium2 Bass kernel for nn_CrossAttention_15006615733765 (raw Bass, no Tile).

Mathematical structure: the reference broadcasts a per-batch context vector
(B, CTX_DIM) to every spatial position before projecting to K/V.  All keys
within a batch are therefore identical, softmax over the key axis is exactly
uniform, and the attention output equals V itself.  The module collapses to

    out[b, c, h, w] = ((context[b] @ Wv) @ Wo + bo)[c]

independent of x, Wq and Wk (exact in infinite precision).  The kernel
computes the two small matmuls on the tensor engine and materializes the
broadcast output shard per core, sharding the 512 output channels across the
8 cores (64 each).

All on-device data is fp16: the 2e-2 rel-err budget dwarfs fp16 rounding
(~5e-4 measured), the Wv stream halves to 768 KB, matmuls avoid the 4x fp32
LOW_HIGH penalty, and the output store halves to 1.18 MB (the host unshard
upcasts to fp32).

Correctness notes learned on HW:
  * start=True (first_mm) clears the whole 2 KiB PSUM bank, racing any
    sibling accumulation group in the same bank.  All matmuls here use
    start=False; the DVE zeroes the result banks at body start instead
    (overwrite-on-clear / accumulate-on-set is then correct regardless of
    stale has_written state).
  * Concurrent PE and DVE PSUM writes (even to different banks) took the
    device down; the HAM warmup matmuls are gated behind the DVE zeroing.
  * A dma_start always fires 16 semaphore increments, one per SDMA engine;
    for transfers with <16 descriptors the excess increments are padding
    that can land BEFORE the data descriptors on other engines.  Every
    gated load here therefore spans all 128 partitions (selector identity,
    bias and ones columns are embedded in the one cw tensor).

Performance notes (from per-instruction NTFF traces):
  * exec time ~= (time the replicated output row is ready) + ~8.5 us: the
    walrus epilogue (per-engine reset of ~200 semaphores after the exit
    rendezvous, ~7.5 us) dominates the tail and fully hides the output
    store, so the only lever is reaching the store issue earlier.
  * Wv is loaded in two 384 KB slabs (3 KiB descriptors) on the two HWDGE
    queues; 1 KiB descriptors measured ~160 GB/s aggregate, 3 KiB ~255.
  * SWDGE (gpsimd) measured ~20-30 GB/s for the consts load and gated
    stage 1 by ~1 us; the consts now ride the scalar HWDGE queue first.
"""

import numpy as np

import concourse.bacc as bacc
import concourse.mybir as mybir
from concourse.bass_utils import run_bass_kernel_spmd

B, DIM, CTX_DIM = 4, 512, 768
H = W = 48
NPOS = H * W
NCORES = 8
CPC = DIM // NCORES  # 64 output channels per core
P = 128
KC = CTX_DIM // P  # 6 contraction chunks for stage 1
KD = DIM // P      # 4 contraction chunks for stage 2
ROW = B * CPC      # 256: one output row (all batches) per position
NDUP = 3           # replicated rows per partition -> 1.5 KiB descriptors
NREP = NPOS // (NDUP * P)  # 6 descriptor groups
F32 = mybir.dt.float32
F16 = mybir.dt.float16

# column offsets inside the packed consts tensor cw [P, CWN]
CTX0 = 0                  # ctx chunks: cw[p, CTX0 + k*B + b] = context[b, k*128+p]
WO0 = CTX0 + KC * B       # Wo slice:   cw[p, WO0 + m*CPC + c] = Wo[m*128+p, cols_i]
BO0 = WO0 + KD * CPC      # bias row:   cw[0, BO0 + c] = bo[c] (partition 0 only)
ONE0 = BO0 + CPC          # ones:       cw[p, ONE0 + j] = 1.0
SEL0 = ONE0 + B           # selector:   cw[r, SEL0 + b] = (r == b), r < B
CWN = SEL0 + B

_CACHE: dict = {}


def _build_nc():
    nc = bacc.Bacc("TRN2", target_bir_lowering=False, debug=False, num_devices=NCORES)

    wvc = nc.dram_tensor("wvc", [P, KC * KD, P], F16, kind="ExternalInput")
    cwc = nc.dram_tensor("cwc", [P, CWN], F16, kind="ExternalInput")
    outd = nc.dram_tensor("outd", [NPOS, ROW], F16, kind="ExternalOutput")

    wv_sb = nc.alloc_sbuf_tensor("wv_sb", [P, KC * KD, P], F16).ap()
    cw_sb = nc.alloc_sbuf_tensor("cw_sb", [P, CWN], F16).ap()
    y1T_sb = nc.alloc_sbuf_tensor("y1T_sb", [P, KD, B], F16).ap()
    o4_sb = nc.alloc_sbuf_tensor("o4_sb", [B, CPC], F16).ap()
    rep_sb = nc.alloc_sbuf_tensor("rep_sb", [P, NDUP, ROW], F16).ap()
    warm_sb = nc.alloc_sbuf_tensor("warm_sb", [P, 512], F16).ap()

    py1T = nc.alloc_psum_tensor("py1T", [P, KD, B], F32).ap()
    po = nc.alloc_psum_tensor("po", [B, CPC], F32).ap()
    prep = nc.alloc_psum_tensor("prep", [P, B, CPC], F32).ap()
    pwarm = nc.alloc_psum_tensor("pwarm", [P, 512], F32).ap()

    from contextlib import ExitStack

    with ExitStack() as stack:
        s_wvs = stack.enter_context(nc.semaphore("s_wvs"))
        s_wvc = stack.enter_context(nc.semaphore("s_wvc"))
        s_pz = stack.enter_context(nc.semaphore("s_pz"))
        s_mm = stack.enter_context(nc.semaphore("s_mm"))
        s_y1 = stack.enter_context(nc.semaphore("s_y1"))
        s_o4 = stack.enter_context(nc.semaphore("s_o4"))
        s_rep = stack.enter_context(nc.semaphore("s_rep"))
        s_out = stack.enter_context(nc.semaphore("s_out"))

        # output view: pos = r*(P*NDUP) + p*NDUP + d; each partition writes
        # NREP contiguous 1.5 KiB chunks (NDUP consecutive 512 B rows).
        HREP = NREP // 2

        with nc.Block() as block:

            @block.sync
            def _(sync):
                sync.dma_start(
                    out=wv_sb[:, :KC * KD // 2, :], in_=wvc[:, :KC * KD // 2, :]
                ).then_inc(s_wvs, 16)
                sync.wait_ge(s_rep, 1)
                out_view = outd.rearrange("(r p d) n -> p r (d n)", p=P, d=NDUP)
                src_view = (
                    rep_sb.rearrange("p d n -> p (d n)")[:, None, :]
                    .broadcast_to((P, HREP, NDUP * ROW))
                )
                # No completion wait: the block-exit DRAIN on the issuing
                # engines waits for the HWDGE queues, so the semaphore-reset
                # epilogue overlaps the transfer.
                sync.dma_start(
                    out=out_view[:, :HREP, :], in_=src_view
                ).then_inc(s_out, 16)

            @block.scalar
            def _(scalar):
                scalar.dma_start(out=cw_sb[:], in_=cwc[:]).then_inc(s_wvc, 16)
                scalar.dma_start(
                    out=wv_sb[:, KC * KD // 2:, :], in_=wvc[:, KC * KD // 2:, :]
                ).then_inc(s_wvc, 16)
                scalar.wait_ge(s_rep, 1)
                out_view = outd.rearrange("(r p d) n -> p r (d n)", p=P, d=NDUP)
                src_view = (
                    rep_sb.rearrange("p d n -> p (d n)")[:, None, :]
                    .broadcast_to((P, HREP, NDUP * ROW))
                )
                scalar.dma_start(
                    out=out_view[:, HREP:, :], in_=src_view
                ).then_inc(s_out, 16)

            @block.tensor
            def _(tensor):
                tensor.wait_ge(s_pz, 1)
                # HAM warmup: dummy matmuls on scratch SBUF keep the PE busy
                # so the 1.2->2.4 GHz unthrottle fires while stage 1 is
                # still load-gated (gated on s_pz: concurrent PE/DVE PSUM
                # writes are fatal).
                NWARM = 3
                for _w in range(NWARM):
                    nc.tensor.matmul(
                        pwarm[:],
                        warm_sb[:, 0:128],
                        warm_sb[:],
                        start=(_w == 0),
                        stop=(_w == NWARM - 1),
                    )

                # bias into po ahead of stage 2: po[b, :] += 1 * bo
                tensor.wait_ge(s_wvc, 16)
                nc.tensor.matmul(
                    po[:],
                    cw_sb[0:1, ONE0:ONE0 + B],
                    cw_sb[0:1, BO0:BO0 + CPC],
                    start=False,
                    stop=False,
                    skip_group_check=True,
                )

                # stage 1: y1T[m*128+p, b] += Wv[k-chunk, m-chunk].T @ ctx
                # chunk.  Produces y1 already transposed -> no PE transposes.
                ins = None
                for k in range(KC):
                    if k == 0:
                        tensor.wait_ge(s_wvs, 16)
                    elif k == KC // 2:
                        tensor.wait_ge(s_wvc, 32)
                    for m in range(KD):
                        ins = nc.tensor.matmul(
                            py1T[:, m, :],
                            wv_sb[:, k * KD + m, :],
                            cw_sb[:, CTX0 + k * B:CTX0 + (k + 1) * B],
                            start=False,
                            stop=(k == KC - 1),
                            skip_group_check=True,
                        )
                ins.then_inc(s_mm, 1)

                # stage 2: y[b, c] += y1T[m-chunk].T @ Wo[m-chunk, cols]
                tensor.wait_ge(s_y1, 1)
                for m in range(KD):
                    ins = nc.tensor.matmul(
                        po[:],
                        y1T_sb[:, m, :],
                        cw_sb[:, WO0 + m * CPC:WO0 + (m + 1) * CPC],
                        start=False,
                        stop=(m == KD - 1),
                        skip_group_check=True,
                    )
                ins.then_inc(s_mm, 1)

                # selector broadcast: prep[p, b, :] = (y[b, :] + bo) for all
                # p; lhsT is a stride-0 broadcast of the identity column.
                tensor.wait_ge(s_o4, 1)
                for b in range(B):
                    ins = nc.tensor.matmul(
                        prep[:, b, :],
                        cw_sb[0:B, SEL0 + b:SEL0 + b + 1].broadcast_to((B, P)),
                        o4_sb[:, :],
                        start=False,
                        stop=True,
                        skip_group_check=True,
                    )
                ins.then_inc(s_mm, 1)

            @block.vector
            def _(vector):
                # Zero the PSUM result banks so the PE matmuls never need
                # start=True (whose whole-bank clear races sibling groups).
                nc.vector.memset(py1T[:], 0.0)
                nc.vector.memset(po[:], 0.0)
                nc.vector.memset(prep[:], 0.0).then_inc(s_pz, 1)
                vector.wait_ge(s_mm, 1)
                nc.vector.tensor_copy(y1T_sb[:], py1T[:]).then_inc(s_y1, 1)
                vector.wait_ge(s_mm, 2)
                nc.vector.tensor_copy(o4_sb[:], po[:]).then_inc(s_o4, 1)
                vector.wait_ge(s_mm, 3)
                flat = prep[:].rearrange("p b c -> p (b c)")
                nc.vector.tensor_copy(rep_sb[:, 0, :], flat)
                # replicas 1..NDUP-1 in one SBUF->SBUF copy (broadcast source)
                nc.vector.tensor_copy(
                    rep_sb[:, 1:, :],
                    rep_sb[:, 0:1, :].broadcast_to((P, NDUP - 1, ROW)),
                ).then_inc(s_rep, 2)

    nc.compile()
    return nc


def _get_nc():
    if "nc" not in _CACHE:
        _CACHE["nc"] = _build_nc()
    return _CACHE["nc"]


def _prepare_in_maps(context, Wv, Wo, bo):
    context = np.asarray(context, dtype=np.float32)
    Wv = np.asarray(Wv, dtype=np.float32)
    Wo = np.asarray(Wo, dtype=np.float32)
    bo = np.asarray(bo, dtype=np.float32)

    # wvc[p, k*KD+m, c] = Wv[k*128+p, m*128+c]  (stationary operands)
    wvc = np.ascontiguousarray(
        Wv.astype(np.float16).reshape(KC, P, KD, P).transpose(1, 0, 2, 3)
        .reshape(P, KC * KD, P)
    )
    # ctx chunks: cw[p, CTX0 + k*B + b] = context[b, k*128+p]
    ctxc = (
        context.astype(np.float16).T.reshape(KC, P, B).transpose(1, 0, 2)
        .reshape(P, KC * B)
    )
    wo16 = Wo.astype(np.float16)
    bo16 = bo.astype(np.float16)

    in_maps = []
    for i in range(NCORES):
        cw = np.zeros((P, CWN), dtype=np.float16)
        cw[:, CTX0:CTX0 + KC * B] = ctxc
        # Wo slice: cw[p, WO0 + m*CPC + c] = Wo[m*128+p, i*CPC+c]
        cw[:, WO0:WO0 + KD * CPC] = (
            wo16[:, i * CPC:(i + 1) * CPC].reshape(KD, P, CPC)
            .transpose(1, 0, 2).reshape(P, KD * CPC)
        )
        cw[0, BO0:BO0 + CPC] = bo16[i * CPC:(i + 1) * CPC]
        cw[:, ONE0:ONE0 + B] = 1.0
        for b in range(B):
            cw[b, SEL0 + b] = 1.0
        in_maps.append({"wvc": wvc, "cwc": cw})
    return in_maps


def _unshard(results):
    shards = np.stack([r["outd"] for r in results], axis=0)
    shards = shards.reshape(NCORES, NPOS, B, CPC)
    out = shards.transpose(2, 0, 3, 1).reshape(B, DIM, H, W)
    return np.ascontiguousarray(out.astype(np.float32))


def kernel(x, context, Wq, Wk, Wv, Wo, bo):
    del x, Wq, Wk
    nc = _get_nc()
    in_maps = _prepare_in_maps(context, Wv, Wo, bo)
    results = run_bass_kernel_spmd(nc, in_maps, list(range(NCORES))).results
    return _unshard(results)


# revision 28
# speedup vs baseline: 1.0896x; 1.0395x over previous
"""Trainium2 Bass kernel for nn_CrossAttention_15006615733765 (raw Bass, no Tile).

Mathematical structure: the reference broadcasts a per-batch context vector
(B, CTX_DIM) to every spatial position before projecting to K/V.  All keys
within a batch are therefore identical, softmax over the key axis is exactly
uniform, and the attention output equals V itself.  The module collapses to

    out[b, c, h, w] = ((context[b] @ Wv) @ Wo + bo)[c]

independent of x, Wq and Wk (exact in infinite precision).  The kernel
computes the two small matmuls on the tensor engine and materializes the
broadcast output shard per core, sharding the 512 output channels across the
8 cores (64 each).

All on-device data is fp16: the 2e-2 rel-err budget dwarfs fp16 rounding
(~5e-4 measured), the Wv stream halves to 768 KB, matmuls avoid the 4x fp32
LOW_HIGH penalty, and the output store halves to 1.18 MB (the host unshard
upcasts to fp32).

Correctness notes learned on HW:
  * start=True (first_mm) clears the whole 2 KiB PSUM bank, racing any
    sibling accumulation group in the same bank.  All matmuls here use
    start=False; the DVE zeroes the result banks at body start instead
    (overwrite-on-clear / accumulate-on-set is then correct regardless of
    stale has_written state).
  * Concurrent PE and DVE PSUM writes (even to different banks) took the
    device down; the HAM warmup matmuls are gated behind the DVE zeroing.
  * A dma_start always fires 16 semaphore increments, one per SDMA engine;
    for transfers with <16 descriptors the excess increments are padding
    that can land BEFORE the data descriptors on other engines.  Every
    gated load here therefore spans all 128 partitions (selector identity,
    bias and ones columns are embedded in the one cw tensor).

Performance notes (from per-instruction NTFF traces):
  * exec time ~= (replicated output row ready) + ~8.7 us: the framework
    NEFF wrapper ends with a per-engine reset of ~200 semaphores after the
    exit rendezvous (~7.5 us) which fully hides the output store, so the
    only lever is reaching the store issue earlier.
  * Wv is loaded in four column-chunk slabs (1.5 KiB descriptors) split
    over the two HWDGE queues, with the consts tensor first on the sync
    queue (the scalar queue's first byte lands ~1-2 us later than sync's).
    Column-major slabs let stage-2 matmuls and the y1T copies for finished
    column chunks overlap the remaining load.
  * SWDGE (gpsimd) measured ~20-30 GB/s and a late start for the consts
    load; everything rides the two HWDGE queues.
"""

import numpy as np

import concourse.bacc as bacc
import concourse.mybir as mybir
from concourse.bass_utils import run_bass_kernel_spmd

B, DIM, CTX_DIM = 4, 512, 768
H = W = 48
NPOS = H * W
NCORES = 8
CPC = DIM // NCORES  # 64 output channels per core
P = 128
KC = CTX_DIM // P  # 6 contraction chunks for stage 1
KD = DIM // P      # 4 column chunks (stage-2 contraction)
ROW = B * CPC      # 256: one output row (all batches) per position
NDUP = 3           # replicated rows per partition -> 1.5 KiB descriptors
NREP = NPOS // (NDUP * P)  # 6 descriptor groups
F32 = mybir.dt.float32
F16 = mybir.dt.float16

# column offsets inside the packed consts tensor cw [P, CWN]
CTX0 = 0                  # ctx chunks: cw[p, CTX0 + k*B + b] = context[b, k*128+p]
WO0 = CTX0 + KC * B       # Wo slice:   cw[p, WO0 + m*CPC + c] = Wo[m*128+p, cols_i]
BO0 = WO0 + KD * CPC      # bias row:   cw[0, BO0 + c] = bo[c] (partition 0 only)
ONE0 = BO0 + CPC          # ones:       cw[p, ONE0 + j] = 1.0
SEL0 = ONE0 + B           # selector:   cw[r, SEL0 + b] = (r == b), r < B
CWN = SEL0 + B

# processing order of the Wv column slabs: sync carries cw, m0, m1 and
# scalar carries m2, m3; consume in expected arrival order.
M_ORDER = (0, 2, 1, 3)
M_GATE = {0: ("s", 32), 1: ("s", 48), 2: ("c", 16), 3: ("c", 32)}

_CACHE: dict = {}


def _build_nc():
    nc = bacc.Bacc("TRN2", target_bir_lowering=False, debug=False, num_devices=NCORES)

    # wvc[p, m*KC + k, c] = Wv[k*128+p, m*128+c]  (column-major slabs)
    wvc = nc.dram_tensor("wvc", [P, KD * KC, P], F16, kind="ExternalInput")
    cwc = nc.dram_tensor("cwc", [P, CWN], F16, kind="ExternalInput")
    outd = nc.dram_tensor("outd", [NPOS, ROW], F16, kind="ExternalOutput")

    wv_sb = nc.alloc_sbuf_tensor("wv_sb", [P, KD * KC, P], F16).ap()
    cw_sb = nc.alloc_sbuf_tensor("cw_sb", [P, CWN], F16).ap()
    y1T_sb = nc.alloc_sbuf_tensor("y1T_sb", [P, KD, B], F16).ap()
    o4_sb = nc.alloc_sbuf_tensor("o4_sb", [B, CPC], F16).ap()
    rep_sb = nc.alloc_sbuf_tensor("rep_sb", [P, NDUP, ROW], F16).ap()
    warm_sb = nc.alloc_sbuf_tensor("warm_sb", [P, 512], F16).ap()

    py1T = [nc.alloc_psum_tensor(f"py1T{m}", [P, B], F32).ap() for m in range(KD)]
    po = nc.alloc_psum_tensor("po", [B, CPC], F32).ap()
    prep = nc.alloc_psum_tensor("prep", [P, B, CPC], F32).ap()
    pwarm = nc.alloc_psum_tensor("pwarm", [P, 512], F32).ap()

    from contextlib import ExitStack

    with ExitStack() as stack:
        s_ws = stack.enter_context(nc.semaphore("s_ws"))
        s_wc = stack.enter_context(nc.semaphore("s_wc"))
        s_pz = stack.enter_context(nc.semaphore("s_pz"))
        s_mm = stack.enter_context(nc.semaphore("s_mm"))
        s_y1 = stack.enter_context(nc.semaphore("s_y1"))
        s_po = stack.enter_context(nc.semaphore("s_po"))
        s_o4 = stack.enter_context(nc.semaphore("s_o4"))
        s_sel = stack.enter_context(nc.semaphore("s_sel"))
        s_rep = stack.enter_context(nc.semaphore("s_rep"))
        s_out = stack.enter_context(nc.semaphore("s_out"))

        HREP = NREP // 2

        with nc.Block() as block:

            @block.sync
            def _(sync):
                sync.dma_start(out=cw_sb[:], in_=cwc[:]).then_inc(s_ws, 16)
                for m in (0, 1):
                    sync.dma_start(
                        out=wv_sb[:, m * KC:(m + 1) * KC, :],
                        in_=wvc[:, m * KC:(m + 1) * KC, :],
                    ).then_inc(s_ws, 16)
                sync.wait_ge(s_rep, 1)
                out_view = outd.rearrange("(r p d) n -> p r (d n)", p=P, d=NDUP)
                src_view = (
                    rep_sb.rearrange("p d n -> p (d n)")[:, None, :]
                    .broadcast_to((P, HREP, NDUP * ROW))
                )
                # No completion wait: the block-exit DRAIN on the issuing
                # engines waits for the HWDGE queues, so the semaphore-reset
                # epilogue overlaps the transfer.
                sync.dma_start(
                    out=out_view[:, :HREP, :], in_=src_view
                ).then_inc(s_out, 16)

            @block.scalar
            def _(scalar):
                for m in (2, 3):
                    scalar.dma_start(
                        out=wv_sb[:, m * KC:(m + 1) * KC, :],
                        in_=wvc[:, m * KC:(m + 1) * KC, :],
                    ).then_inc(s_wc, 16)
                scalar.wait_ge(s_rep, 1)
                out_view = outd.rearrange("(r p d) n -> p r (d n)", p=P, d=NDUP)
                src_view = (
                    rep_sb.rearrange("p d n -> p (d n)")[:, None, :]
                    .broadcast_to((P, HREP, NDUP * ROW))
                )
                scalar.dma_start(
                    out=out_view[:, HREP:, :], in_=src_view
                ).then_inc(s_out, 16)

            @block.tensor
            def _(tensor):
                tensor.wait_ge(s_pz, 1)
                # HAM warmup: dummy matmuls on scratch SBUF keep the PE busy
                # so the 1.2->2.4 GHz unthrottle fires while stage 1 is
                # still load-gated (gated on s_pz: concurrent PE/DVE PSUM
                # writes are fatal).
                NWARM = 3
                for _w in range(NWARM):
                    nc.tensor.matmul(
                        pwarm[:],
                        warm_sb[:, 0:128],
                        warm_sb[:],
                        start=(_w == 0),
                        stop=(_w == NWARM - 1),
                    )

                # bias into po ahead of stage 2: po[b, :] += 1 * bo
                tensor.wait_ge(s_ws, 16)
                nc.tensor.matmul(
                    po[:],
                    cw_sb[0:1, ONE0:ONE0 + B],
                    cw_sb[0:1, BO0:BO0 + CPC],
                    start=False,
                    stop=False,
                    skip_group_check=True,
                )

                # stage 1 per column slab m: y1T[m*128+p, b] += sum_k
                # Wv[k, m].T @ ctx[k]; stage-2 MMs for finished slabs are
                # interleaved so they overlap the remaining load.
                def stage1(m):
                    eng, val = M_GATE[m]
                    tensor.wait_ge(s_ws if eng == "s" else s_wc, val)
                    ins = None
                    for k in range(KC):
                        ins = nc.tensor.matmul(
                            py1T[m][:],
                            wv_sb[:, m * KC + k, :],
                            cw_sb[:, CTX0 + k * B:CTX0 + (k + 1) * B],
                            start=False,
                            stop=(k == KC - 1),
                            skip_group_check=True,
                        )
                    ins.then_inc(s_mm, 1)

                def stage2(i, m):
                    tensor.wait_ge(s_y1, i + 1)
                    ins = nc.tensor.matmul(
                        po[:],
                        y1T_sb[:, m, :],
                        cw_sb[:, WO0 + m * CPC:WO0 + (m + 1) * CPC],
                        start=False,
                        stop=(i == KD - 1),
                        skip_group_check=True,
                    )
                    if i == KD - 1:
                        ins.then_inc(s_po, 1)

                stage1(M_ORDER[0])
                stage1(M_ORDER[1])
                stage2(0, M_ORDER[0])
                stage1(M_ORDER[2])
                stage2(1, M_ORDER[1])
                stage1(M_ORDER[3])
                stage2(2, M_ORDER[2])
                stage2(3, M_ORDER[3])

                # selector broadcast: prep[p, b, :] = (y[b, :] + bo) for all
                # p; lhsT is a stride-0 broadcast of the identity column.
                tensor.wait_ge(s_o4, 1)
                for b in range(B):
                    ins = nc.tensor.matmul(
                        prep[:, b, :],
                        cw_sb[0:B, SEL0 + b:SEL0 + b + 1].broadcast_to((B, P)),
                        o4_sb[:, :],
                        start=False,
                        stop=True,
                        skip_group_check=True,
                    )
                ins.then_inc(s_sel, 1)

            @block.vector
            def _(vector):
                # Zero the PSUM result banks so the PE matmuls never need
                # start=True (whose whole-bank clear races sibling groups).
                for m in range(KD):
                    nc.vector.memset(py1T[m][:], 0.0)
                nc.vector.memset(po[:], 0.0)
                nc.vector.memset(prep[:], 0.0).then_inc(s_pz, 1)
                for i, m in enumerate(M_ORDER):
                    vector.wait_ge(s_mm, i + 1)
                    nc.vector.tensor_copy(
                        y1T_sb[:, m, :], py1T[m][:]
                    ).then_inc(s_y1, 1)
                vector.wait_ge(s_po, 1)
                nc.vector.tensor_copy(o4_sb[:], po[:]).then_inc(s_o4, 1)
                vector.wait_ge(s_sel, 1)
                flat = prep[:].rearrange("p b c -> p (b c)")
                nc.vector.tensor_copy(rep_sb[:, 0, :], flat)
                # replicas 1..NDUP-1 in one SBUF->SBUF copy (broadcast source)
                nc.vector.tensor_copy(
                    rep_sb[:, 1:, :],
                    rep_sb[:, 0:1, :].broadcast_to((P, NDUP - 1, ROW)),
                ).then_inc(s_rep, 2)

    nc.compile()
    return nc


def _get_nc():
    if "nc" not in _CACHE:
        _CACHE["nc"] = _build_nc()
    return _CACHE["nc"]


def _prepare_in_maps(context, Wv, Wo, bo):
    context = np.asarray(context, dtype=np.float32)
    Wv = np.asarray(Wv, dtype=np.float32)
    Wo = np.asarray(Wo, dtype=np.float32)
    bo = np.asarray(bo, dtype=np.float32)

    # wvc[p, m*KC+k, c] = Wv[k*128+p, m*128+c]  (column-major slabs)
    wvc = np.ascontiguousarray(
        Wv.astype(np.float16).reshape(KC, P, KD, P).transpose(1, 2, 0, 3)
        .reshape(P, KD * KC, P)
    )
    # ctx chunks: cw[p, CTX0 + k*B + b] = context[b, k*128+p]
    ctxc = (
        context.astype(np.float16).T.reshape(KC, P, B).transpose(1, 0, 2)
        .reshape(P, KC * B)
    )
    wo16 = Wo.astype(np.float16)
    bo16 = bo.astype(np.float16)

    in_maps = []
    for i in range(NCORES):
        cw = np.zeros((P, CWN), dtype=np.float16)
        cw[:, CTX0:CTX0 + KC * B] = ctxc
        # Wo slice: cw[p, WO0 + m*CPC + c] = Wo[m*128+p, i*CPC+c]
        cw[:, WO0:WO0 + KD * CPC] = (
            wo16[:, i * CPC:(i + 1) * CPC].reshape(KD, P, CPC)
            .transpose(1, 0, 2).reshape(P, KD * CPC)
        )
        cw[0, BO0:BO0 + CPC] = bo16[i * CPC:(i + 1) * CPC]
        cw[:, ONE0:ONE0 + B] = 1.0
        for b in range(B):
            cw[b, SEL0 + b] = 1.0
        in_maps.append({"wvc": wvc, "cwc": cw})
    return in_maps


def _unshard(results):
    shards = np.stack([r["outd"] for r in results], axis=0)
    shards = shards.reshape(NCORES, NPOS, B, CPC)
    out = shards.transpose(2, 0, 3, 1).reshape(B, DIM, H, W)
    return np.ascontiguousarray(out.astype(np.float32))


def kernel(x, context, Wq, Wk, Wv, Wo, bo):
    del x, Wq, Wk
    nc = _get_nc()
    in_maps = _prepare_in_maps(context, Wv, Wo, bo)
    results = run_bass_kernel_spmd(nc, in_maps, list(range(NCORES))).results
    return _unshard(results)
